# revision 21
# baseline (speedup 1.0000x reference)
"""MoE routing kernel for 8 Trainium2 NeuronCores.

Strategy (expert-parallel, 3 launches; host does only data movement):
  L1  router   : data-parallel over tokens. Exact-fp32 gate matmul in
                 token-partition orientation (out free dim = 8 experts, so
                 the fp32 4x penalty is negligible), top-2 via DVE
                 max/max_index on logits (sigmoid monotone; bias path when
                 expert_bias != 0), per-tile sigmoid, batched output DMAs.
  L2  experts  : one expert per core. Host gathers + transposes that
                 expert's token rows to [D, CAP] bf16 and replicates the
                 gate row to [128, CAP]; device pre-scales by gate on DVE,
                 runs the GLU MLP as pure bf16 GEMMs (no on-device
                 transposes or gathers), and fuses the post-scale into the
                 PSUM->bf16 drain. Weights arrive as per-m-block DMAs in
                 m-major host layout so the first GEMM starts ~6us in; the
                 w2 GEMM of tile t-1 is interleaved inside the w1/w3 GEMMs
                 of tile t so the PE never stalls on the gT latency.
  L3  combine  : data-parallel over token slices. Shared-expert GLU MLP in
                 bf16, combine = two DVE adds of host-retransposed routed
                 contributions (AT/BT, [D, TPC] bf16) directly on the w2
                 PSUM output; result stays [D, TPC] f32 (host transposes
                 back), output drained in half-chunks to overlap the final
                 DMA with compute.
"""
import sys
sys.path.insert(0, '/opt/trn_rl_repo')

import numpy as np
import ml_dtypes

import concourse.bacc as bacc
import concourse.mybir as mybir
import concourse.tile as tile
from concourse.bass_utils import run_bass_kernel_spmd

F32 = mybir.dt.float32
BF16 = mybir.dt.bfloat16
U32 = mybir.dt.uint32
AF = mybir.ActivationFunctionType
ALU = mybir.AluOpType
NPBF16 = ml_dtypes.bfloat16

NCORES = 8
E = 8           # experts
K = 2           # top-k
D = 1024
H = 1024
T = 8192        # total tokens (B*S)
TPC = T // NCORES   # tokens per core (router / combine slices)


def _mmajor(wT):
    """[D, H] f32 -> [8(m), 128(p), 8(k), 128(j)] bf16 contiguous, so a
    per-m-block DMA moves 2KB-contiguous rows: w[m, p, k, j] = wT[k*128+p,
    m*128+j]."""
    return np.ascontiguousarray(
        wT.reshape(8, 128, 8, 128).transpose(2, 1, 0, 3).astype(NPBF16))


# --------------------------------------------------------------- L1: router
def build_l1(bias_vals):
    nc = bacc.Bacc("TRN2", target_bir_lowering=False, debug=False,
                   num_devices=NCORES)
    xT = nc.dram_tensor("xT", [D, TPC], F32, kind="ExternalInput").ap()
    gwc = nc.dram_tensor("gwc", [128, 8, E], F32, kind="ExternalInput").ap()
    gates_o = nc.dram_tensor("gates", [TPC, K], F32, kind="ExternalOutput").ap()
    idx_o = nc.dram_tensor("idx", [TPC, K], U32, kind="ExternalOutput").ap()
    bias_zero = all(float(b) == 0.0 for b in bias_vals)
    NT = TPC // 128

    with tile.TileContext(nc) as tc:
        with tc.tile_pool(name="pin", bufs=1) as pin, \
             tc.tile_pool(name="pps", bufs=4, space="PSUM") as pps, \
             tc.tile_pool(name="pwk", bufs=4) as pwk:
            xT_sb = pin.tile([128, NT, 8, 128], F32)
            gw_sb = pin.tile([128, 8, E], F32)
            for t in range(NT):
                nc.sync.dma_start(
                    xT_sb[:, t, :, :],
                    xT[:, t*128:(t+1)*128].rearrange("(k p) n -> p k n", p=128))
                if t == 0:
                    nc.sync.dma_start(gw_sb[:], gwc[:])
            gout = pin.tile([128, NT, K], F32)
            icoll = pin.tile([128, NT, K], U32)

            for t in range(NT):
                ps = pps.tile([128, E], F32, tag="ps")
                for k in range(8):
                    nc.tensor.matmul(ps[:], xT_sb[:, t, k, :], gw_sb[:, k, :],
                                     start=(k == 0), stop=(k == 7))
                sel = pwk.tile([128, E], F32, tag="sel")
                if bias_zero:
                    # selection key = logits (sigmoid monotone, bias 0)
                    nc.vector.tensor_copy(sel[:], ps[:])
                else:
                    # selection key = sigmoid(logits) + bias
                    nc.scalar.activation(sel[:], ps[:], AF.Sigmoid)
                    for e in range(E):
                        if float(bias_vals[e]) != 0.0:
                            nc.vector.tensor_scalar_add(
                                sel[:, e:e+1], sel[:, e:e+1], float(bias_vals[e]))
                top8 = pwk.tile([128, 8], F32, tag="top8")
                nc.vector.max(top8[:], sel[:])
                idx8 = pwk.tile([128, 8], U32, tag="idx8")
                nc.vector.max_index(idx8[:], top8[:], sel[:])
                nc.vector.tensor_copy(icoll[:, t, :], idx8[:, 0:K])
                if bias_zero:
                    nc.scalar.activation(gout[:, t, :], top8[:, 0:K], AF.Sigmoid)
                else:
                    # true score = (sigmoid+bias) - bias[selected]
                    nc.vector.tensor_copy(gout[:, t, :], top8[:, 0:K])
                    idxf = pwk.tile([128, K], F32, tag="idxf")
                    nc.vector.tensor_copy(idxf[:], idx8[:, 0:K])
                    for e in range(E):
                        if float(bias_vals[e]) == 0.0:
                            continue
                        m = pwk.tile([128, K], F32, tag="msk")
                        nc.vector.tensor_scalar(m[:], idxf[:], float(e), None,
                                                op0=ALU.is_equal)
                        nc.vector.tensor_scalar_mul(m[:], m[:],
                                                    -float(bias_vals[e]))
                        nc.vector.tensor_add(gout[:, t, :], gout[:, t, :], m[:])
                if t % 4 == 3:
                    cs = slice((t-3)*128, (t+1)*128)
                    nc.sync.dma_start(
                        idx_o[cs, :].rearrange("(t p) k -> p t k", p=128),
                        icoll[:, t-3:t+1, :])
                    nc.sync.dma_start(
                        gates_o[cs, :].rearrange("(t p) k -> p t k", p=128),
                        gout[:, t-3:t+1, :])
    nc.compile()
    return nc


# -------------------------------------------------------------- L2: experts
def build_l2(cap):
    nc = bacc.Bacc("TRN2", target_bir_lowering=False, debug=False,
                   num_devices=NCORES)
    xgT = nc.dram_tensor("xgT", [D, cap], BF16, kind="ExternalInput").ap()
    gbr = nc.dram_tensor("gbr", [128, cap], BF16, kind="ExternalInput").ap()
    w1h = nc.dram_tensor("w1h", [8, 128, 8, 128], BF16, kind="ExternalInput").ap()
    w3h = nc.dram_tensor("w3h", [8, 128, 8, 128], BF16, kind="ExternalInput").ap()
    w2T = nc.dram_tensor("w2T", [H, D], BF16, kind="ExternalInput").ap()
    yT_o = nc.dram_tensor("yT", [D, cap], BF16, kind="ExternalOutput").ap()

    # first tile is 256 wide so the opening xgT DMA (the startup critical
    # path) is half-size; remainder lands on the last tile
    tws = [256]
    left = cap - 256
    while left > 512:
        tws.append(512)
        left -= 512
    tws.append(left)
    ntiles = len(tws)
    starts = [0]
    for w in tws[:-1]:
        starts.append(starts[-1] + w)

    with tile.TileContext(nc) as tc:
        with tc.tile_pool(name="pin", bufs=1) as pin, \
             tc.tile_pool(name="pxg", bufs=2) as pxg, \
             tc.tile_pool(name="pxs", bufs=2) as pxs, \
             tc.tile_pool(name="pgt", bufs=2) as pgt, \
             tc.tile_pool(name="pwk", bufs=2) as pwk, \
             tc.tile_pool(name="pyo", bufs=2) as pyo, \
             tc.tile_pool(name="pps", bufs=1, space="PSUM") as pps:
            gb_sb = pin.tile([128, cap], BF16)

            def load(t):
                tw = tws[t]
                cs = slice(starts[t], starts[t] + tw)
                nc.sync.dma_start(gb_sb[:, cs], gbr[:, cs])
                xg = pxg.tile([128, 8, 512], BF16, tag="xg")
                nc.sync.dma_start(
                    xg[:, :, 0:tw],
                    xgT[:, cs].rearrange("(k p) n -> p k n", p=128))
                return xg

            xg0 = load(0)
            # PE p-state warm-up: tiny matmuls on the first-arrived gb chunk
            # keep the PE busy (and the clock ramping) while weights stream
            # in; without this the first ~45 real matmuls are costed at the
            # un-ramped 1.2GHz rate.
            # warm source is an uninitialized SBUF tile: no producer, so the
            # warm-up chain starts immediately instead of waiting on a DMA
            # (values are garbage but land in an unread PSUM bank)
            wsrc = pin.tile([128, 512], BF16)
            nc.vector.memset(wsrc[:], 0.0)
            for _ in range(11):
                wp = pps.tile([128, 512], F32, tag="warm", bufs=1, name="wp")
                nc.tensor.matmul(wp[0:1, :], wsrc[:, 0:1], wsrc[:],
                                 start=True, stop=True)
            # m-major weight layout: per-m-block DMAs with 2KB descriptors so
            # the first h1 GEMM only waits on w1[m=0]; tile-1's load is
            # interleaved mid-stream so the PE (which clears the narrow
            # tile 0 quickly) never waits on it behind the weight queue
            w1r = pin.tile([128, 8, 8, 128], BF16)
            w3r = pin.tile([128, 8, 8, 128], BF16)
            for m in range(4):
                nc.sync.dma_start(w1r[:, m, :, :], w1h[m, :, :, :])
                nc.sync.dma_start(w3r[:, m, :, :], w3h[m, :, :, :])
            xg1 = load(1)
            for m in range(4, 8):
                nc.sync.dma_start(w1r[:, m, :, :], w1h[m, :, :, :])
                nc.sync.dma_start(w3r[:, m, :, :], w3h[m, :, :, :])
            w2r = pin.tile([128, 8, D], BF16)
            nc.sync.dma_start(w2r[:], w2T.rearrange("(m p) d -> p m d", p=128))

            def xscale(t, xg):
                tw = tws[t]
                cs = slice(starts[t], starts[t] + tw)
                xs = pxs.tile([128, 8, 512], BF16, tag="xs")
                for k in range(8):
                    nc.vector.tensor_mul(xs[:, k, 0:tw], xg[:, k, 0:tw],
                                         gb_sb[:, cs])
                return xs

            def hpart(t, xs, ms):
                tw = tws[t]
                gT = gts[t % 2]
                for m in ms:
                    h1 = pps.tile([128, 512], F32, tag="h1", bufs=2)
                    h3 = pps.tile([128, 512], F32, tag="h3", bufs=2)
                    for k in range(8):
                        nc.tensor.matmul(h1[:, 0:tw], w1r[:, m, k, :],
                                         xs[:, k, 0:tw],
                                         start=(k == 0), stop=(k == 7))
                    for k in range(8):
                        nc.tensor.matmul(h3[:, 0:tw], w3r[:, m, k, :],
                                         xs[:, k, 0:tw],
                                         start=(k == 0), stop=(k == 7))
                    s1 = pwk.tile([128, 512], BF16, tag="s1")
                    nc.scalar.activation(s1[:, 0:tw], h1[:, 0:tw], AF.Silu)
                    nc.vector.tensor_mul(gT[:, m, 0:tw], s1[:, 0:tw], h3[:, 0:tw])

            def ypart(t):
                tw = tws[t]
                cs = slice(starts[t], starts[t] + tw)
                last = (t == ntiles - 1)
                gT = gts[t % 2]
                yo = pyo.tile([128, 8, 512], BF16, tag="yo")
                for d in range(8):
                    yp = pps.tile([128, 512], F32, tag="yp", bufs=3)
                    for m in range(8):
                        nc.tensor.matmul(yp[:, 0:tw], w2r[:, m, d*128:(d+1)*128],
                                         gT[:, m, 0:tw],
                                         start=(m == 0), stop=(m == 7))
                    # post-scale by gate fused into the PSUM drain
                    nc.vector.tensor_mul(yo[:, d, 0:tw], yp[:, 0:tw],
                                         gb_sb[:, cs])
                    if last and d % 2 == 1:
                        nc.sync.dma_start(
                            yT_o[(d-1)*128:(d+1)*128, cs].rearrange(
                                "(d p) n -> p d n", p=128),
                            yo[:, d-1:d+1, 0:tw])
                    elif not last and d == 3:
                        nc.sync.dma_start(
                            yT_o[0:512, cs].rearrange(
                                "(d p) n -> p d n", p=128),
                            yo[:, 0:4, 0:tw])
                if not last:
                    nc.sync.dma_start(
                        yT_o[512:1024, cs].rearrange(
                            "(d p) n -> p d n", p=128),
                        yo[:, 4:8, 0:tw])

            gts = [pgt.tile([128, 8, 512], BF16, tag="gTa", name="gTa"),
                   pgt.tile([128, 8, 512], BF16, tag="gTb", name="gTb")]

            xs = xscale(0, xg0)
            hpart(0, xs, range(8))
            for t in range(1, ntiles):
                xg = xg1 if t == 1 else load(t)
                xs = xscale(t, xg)
                # interleave w2 GEMM of tile t-1 inside w1/w3 GEMMs of tile t
                hpart(t, xs, range(4))
                ypart(t - 1)
                hpart(t, xs, range(4, 8))
            ypart(ntiles - 1)
    nc.compile()
    return nc


# ------------------------------------------------------ L3: shared + combine
def build_l3():
    nc = bacc.Bacc("TRN2", target_bir_lowering=False, debug=False,
                   num_devices=NCORES)
    xTr = nc.dram_tensor("xTr", [D, TPC], BF16, kind="ExternalInput").ap()
    sw1h = nc.dram_tensor("sw1h", [8, 128, 8, 128], BF16, kind="ExternalInput").ap()
    sw3h = nc.dram_tensor("sw3h", [8, 128, 8, 128], BF16, kind="ExternalInput").ap()
    sw2T = nc.dram_tensor("sw2T", [H, D], BF16, kind="ExternalInput").ap()
    AT = nc.dram_tensor("AT", [D, TPC], BF16, kind="ExternalInput").ap()
    BT = nc.dram_tensor("BT", [D, TPC], BF16, kind="ExternalInput").ap()
    warm = nc.dram_tensor("warm", [128, 512], BF16, kind="ExternalInput").ap()
    outT_o = nc.dram_tensor("outT", [D, TPC], F32, kind="ExternalOutput").ap()

    nh = TPC // 512

    with tile.TileContext(nc) as tc:
        with tc.tile_pool(name="pin", bufs=1) as pin, \
             tc.tile_pool(name="pgt", bufs=2) as pgt, \
             tc.tile_pool(name="pwk", bufs=2) as pwk, \
             tc.tile_pool(name="pos", bufs=2) as pos, \
             tc.tile_pool(name="pab", bufs=2) as pab, \
             tc.tile_pool(name="pps", bufs=1, space="PSUM") as pps:
            wm_sb = pin.tile([128, 512], BF16)
            nc.sync.dma_start(wm_sb[:], warm[:])
            xT_sb = pin.tile([128, 8, TPC], BF16)
            nc.sync.dma_start(
                xT_sb[:, :, 0:512],
                xTr[:, 0:512].rearrange("(k p) n -> p k n", p=128))
            # PE p-state warm-up (see L2)
            for _ in range(13):
                wp = pps.tile([128, 512], F32, tag="warm", bufs=1, name="wp")
                nc.tensor.matmul(wp[0:1, :], wm_sb[:, 0:1], wm_sb[:],
                                 start=True, stop=True)
            w1r = pin.tile([128, 8, 8, 128], BF16)
            w3r = pin.tile([128, 8, 8, 128], BF16)
            for m in range(8):
                nc.sync.dma_start(w1r[:, m, :, :], sw1h[m, :, :, :])
                nc.sync.dma_start(w3r[:, m, :, :], sw3h[m, :, :, :])
            for hh in range(1, nh):
                nc.sync.dma_start(
                    xT_sb[:, :, hh*512:(hh+1)*512],
                    xTr[:, hh*512:(hh+1)*512].rearrange("(k p) n -> p k n", p=128))
            w2r = pin.tile([128, 8, D], BF16)
            nc.sync.dma_start(w2r[:], sw2T.rearrange("(m p) d -> p m d", p=128))
            at_sb = pin.tile([128, 8, TPC], BF16)
            bt_sb = pin.tile([128, 8, TPC], BF16)
            for hh in range(nh):
                cs = slice(hh*512, (hh+1)*512)
                nc.sync.dma_start(at_sb[:, :, cs],
                                  AT[:, cs].rearrange("(d p) n -> p d n", p=128))
                nc.sync.dma_start(bt_sb[:, :, cs],
                                  BT[:, cs].rearrange("(d p) n -> p d n", p=128))

            gts = [pgt.tile([128, 8, 512], BF16, tag="gTa", name="gTa"),
                   pgt.tile([128, 8, 512], BF16, tag="gTb", name="gTb")]

            def hpart(hh, ms, split_first=False):
                toks = slice(hh*512, (hh+1)*512)
                gT = gts[hh % 2]
                for m in ms:
                    h1 = pps.tile([128, 512], F32, tag="h1", bufs=2)
                    h3 = pps.tile([128, 512], F32, tag="h3", bufs=2)
                    if split_first and m == ms[0]:
                        for half in range(2):
                            cls = slice(half*256, (half+1)*256)
                            for k in range(8):
                                nc.tensor.matmul(
                                    h1[:, cls], w1r[:, m, k, :],
                                    xT_sb[:, k, hh*512+half*256:
                                          hh*512+(half+1)*256],
                                    start=(k == 0), stop=(k == 7))
                    else:
                        for k in range(8):
                            nc.tensor.matmul(h1[:], w1r[:, m, k, :],
                                             xT_sb[:, k, toks],
                                             start=(k == 0), stop=(k == 7))
                    for k in range(8):
                        nc.tensor.matmul(h3[:], w3r[:, m, k, :],
                                         xT_sb[:, k, toks],
                                         start=(k == 0), stop=(k == 7))
                    s1 = pwk.tile([128, 512], BF16, tag="s1")
                    nc.scalar.activation(s1[:], h1[:], AF.Silu)
                    nc.vector.tensor_mul(gT[:, m, :], s1[:], h3[:])

            def absum(hh):
                # A+B pre-sum on DVE slack so the combine is one op per block
                toks = slice(hh*512, (hh+1)*512)
                ab = pab.tile([128, 8, 512], BF16, tag="ab")
                for d in range(8):
                    nc.vector.tensor_add(ab[:, d, :], at_sb[:, d, toks],
                                         bt_sb[:, d, toks])
                return ab

            def ypart(hh, ab):
                toks = slice(hh*512, (hh+1)*512)
                last = (hh == nh - 1)
                gT = gts[hh % 2]
                out_sb = pos.tile([128, 8, 512], F32, tag="os")
                for d in range(8):
                    yp = pps.tile([128, 512], F32, tag="yp", bufs=3)
                    for m in range(8):
                        nc.tensor.matmul(yp[:], w2r[:, m, d*128:(d+1)*128],
                                         gT[:, m, :],
                                         start=(m == 0), stop=(m == 7))
                    # combine: out = shared + (A + B), straight off PSUM
                    nc.vector.scalar_tensor_tensor(
                        out_sb[:, d, :], yp[:], 1.0, ab[:, d, :],
                        op0=ALU.mult, op1=ALU.add)
                    # drain: on the last half, shrink the final chunks so the
                    # tail after the last matmul is just d7's DMA
                    if last:
                        chunk = {1: 0, 3: 2, 5: 4, 6: 6, 7: 7}.get(d)
                    else:
                        chunk = d - 1 if d % 2 == 1 else None
                    if chunk is not None:
                        nc.sync.dma_start(
                            outT_o[chunk*128:(d+1)*128, toks].rearrange(
                                "(d p) n -> p d n", p=128),
                            out_sb[:, chunk:d+1, :])

            hpart(0, range(8), split_first=True)
            ab = absum(0)
            for hh in range(1, nh):
                hpart(hh, range(4))
                ypart(hh - 1, ab)
                hpart(hh, range(4, 8))
                ab = absum(hh)
            ypart(nh - 1, ab)
    nc.compile()
    return nc


_BUILT = {}


def _get(name, builder, *args):
    key = (name,) + tuple(args)
    if key not in _BUILT:
        _BUILT[key] = builder(*args)
    return _BUILT[key], key


def kernel(**inputs):
    x = np.ascontiguousarray(np.asarray(inputs["x"], dtype=np.float32))
    xf = x.reshape(T, D)
    gw = np.asarray(inputs["gate_w"], dtype=np.float32)
    bias = np.asarray(inputs["expert_bias"], dtype=np.float32)
    w1 = np.asarray(inputs["w1"], dtype=np.float32)
    w2 = np.asarray(inputs["w2"], dtype=np.float32)
    w3 = np.asarray(inputs["w3"], dtype=np.float32)
    sw1 = np.asarray(inputs["sw1"], dtype=np.float32)
    sw2 = np.asarray(inputs["sw2"], dtype=np.float32)
    sw3 = np.asarray(inputs["sw3"], dtype=np.float32)

    cores = list(range(NCORES))

    # ---- L1 router ----
    nc1, _ = _get("l1", build_l1, tuple(float(b) for b in bias))
    gwc = np.ascontiguousarray(gw.T.reshape(8, 128, 8).transpose(1, 0, 2))
    in1 = [{"xT": np.ascontiguousarray(xf[c*TPC:(c+1)*TPC].T), "gwc": gwc}
           for c in cores]
    r1 = run_bass_kernel_spmd(nc1, in1, cores).results
    gates = np.concatenate([r["gates"] for r in r1])      # [T, 2]
    sel = np.concatenate([r["idx"] for r in r1])          # [T, 2] uint32

    # ---- host dispatch (pure permutation / layout) ----
    flat_sel = sel.reshape(-1).astype(np.int64)
    order = np.argsort(flat_sel, kind="stable")
    counts = np.bincount(flat_sel, minlength=E)
    offs = np.zeros(E + 1, np.int64)
    np.cumsum(counts, out=offs[1:])
    cap = max(512, int(counts.max()))
    gflat = gates.reshape(-1)
    xf_bf = xf.astype(NPBF16)

    slots_e = [order[offs[e]:offs[e+1]] for e in range(E)]
    in2 = []
    for e in cores:
        n = counts[e]
        slots = slots_e[e]
        toks = slots >> 1
        xg = np.zeros((cap, D), NPBF16)
        xg[:n] = xf_bf[toks]
        gbrow = np.zeros((cap,), NPBF16)
        gbrow[:n] = gflat[slots].astype(NPBF16)
        in2.append({
            "xgT": np.ascontiguousarray(xg.T),
            "gbr": np.ascontiguousarray(np.broadcast_to(gbrow, (128, cap))),
            "w1h": _mmajor(w1[e].T),
            "w3h": _mmajor(w3[e].T),
            "w2T": np.ascontiguousarray(w2[e].T.astype(NPBF16)),
        })

    nc2, _ = _get("l2", build_l2, cap)
    r2 = run_bass_kernel_spmd(nc2, in2, cores).results

    # ---- host re-layout of routed contributions (pure permutation) ----
    ATfull = np.zeros((D, T), NPBF16)
    BTfull = np.zeros((D, T), NPBF16)
    total_valid = 0
    for e in cores:
        n = counts[e]
        slots = slots_e[e]
        toks = slots >> 1
        kk = (slots & 1).astype(bool)
        yT = r2[e]["yT"]                         # [D, cap] bf16
        ATfull[:, toks[~kk]] = yT[:, :n][:, ~kk]
        BTfull[:, toks[kk]] = yT[:, :n][:, kk]
        total_valid += n
    assert total_valid == T * K, f"dropped slots: {total_valid} != {T*K}"

    # ---- L3 shared + combine ----
    nc3, _ = _get("l3", build_l3)
    sw1h = _mmajor(sw1.T)
    sw3h = _mmajor(sw3.T)
    sw2T = np.ascontiguousarray(sw2.T.astype(NPBF16))
    in3 = []
    for c in cores:
        sl = slice(c*TPC, (c+1)*TPC)
        in3.append({
            "xTr": np.ascontiguousarray(xf_bf[sl].T),
            "sw1h": sw1h, "sw3h": sw3h, "sw2T": sw2T,
            "AT": np.ascontiguousarray(ATfull[:, sl]),
            "BT": np.ascontiguousarray(BTfull[:, sl]),
            "warm": np.zeros((128, 512), NPBF16),
        })
    r3 = run_bass_kernel_spmd(nc3, in3, cores).results
    out = np.concatenate([r["outT"].T for r in r3])
    return np.ascontiguousarray(out).reshape(x.shape).astype(
        inputs["x"].dtype, copy=False)


# revision 22
# speedup vs baseline: 1.0025x; 1.0025x over previous
"""MoE routing kernel for 8 Trainium2 NeuronCores.

Strategy (expert-parallel, 3 launches; host does only data movement):
  L1  router   : data-parallel over tokens. Exact-fp32 gate matmul in
                 token-partition orientation (out free dim = 8 experts, so
                 the fp32 4x penalty is negligible), top-2 via DVE
                 max/max_index on logits (sigmoid monotone; bias path when
                 expert_bias != 0), per-tile sigmoid, batched output DMAs.
  L2  experts  : one expert per core. Host gathers + transposes that
                 expert's token rows to [D, CAP] bf16 and replicates the
                 gate row to [128, CAP]; device pre-scales by gate on DVE,
                 runs the GLU MLP as pure bf16 GEMMs (no on-device
                 transposes or gathers), and fuses the post-scale into the
                 PSUM->bf16 drain. Weights arrive as per-m-block DMAs in
                 m-major host layout so the first GEMM starts ~6us in; the
                 w2 GEMM of tile t-1 is interleaved inside the w1/w3 GEMMs
                 of tile t so the PE never stalls on the gT latency.
  L3  combine  : data-parallel over token slices. Shared-expert GLU MLP in
                 bf16, combine = two DVE adds of host-retransposed routed
                 contributions (AT/BT, [D, TPC] bf16) directly on the w2
                 PSUM output; result stays [D, TPC] f32 (host transposes
                 back), output drained in half-chunks to overlap the final
                 DMA with compute.
"""
import sys
sys.path.insert(0, '/opt/trn_rl_repo')

import numpy as np
import ml_dtypes

import concourse.bacc as bacc
import concourse.mybir as mybir
import concourse.tile as tile
from concourse.bass_utils import run_bass_kernel_spmd

F32 = mybir.dt.float32
BF16 = mybir.dt.bfloat16
U32 = mybir.dt.uint32
AF = mybir.ActivationFunctionType
ALU = mybir.AluOpType
NPBF16 = ml_dtypes.bfloat16

NCORES = 8
E = 8           # experts
K = 2           # top-k
D = 1024
H = 1024
T = 8192        # total tokens (B*S)
TPC = T // NCORES   # tokens per core (router / combine slices)


def _mmajor(wT):
    """[D, H] f32 -> [8(m), 128(p), 8(k), 128(j)] bf16 contiguous, so a
    per-m-block DMA moves 2KB-contiguous rows: w[m, p, k, j] = wT[k*128+p,
    m*128+j]."""
    return np.ascontiguousarray(
        wT.reshape(8, 128, 8, 128).transpose(2, 1, 0, 3).astype(NPBF16))


# --------------------------------------------------------------- L1: router
def build_l1(bias_vals):
    nc = bacc.Bacc("TRN2", target_bir_lowering=False, debug=False,
                   num_devices=NCORES)
    xT = nc.dram_tensor("xT", [D, TPC], F32, kind="ExternalInput").ap()
    gwc = nc.dram_tensor("gwc", [128, 8, E], F32, kind="ExternalInput").ap()
    comb_o = nc.dram_tensor("comb", [TPC, 2 * K], U32, kind="ExternalOutput").ap()
    bias_zero = all(float(b) == 0.0 for b in bias_vals)
    NT = TPC // 128

    with tile.TileContext(nc) as tc:
        with tc.tile_pool(name="pin", bufs=1) as pin, \
             tc.tile_pool(name="pps", bufs=4, space="PSUM") as pps, \
             tc.tile_pool(name="pwk", bufs=4) as pwk:
            xT_sb = pin.tile([128, NT, 8, 128], F32)
            gw_sb = pin.tile([128, 8, E], F32)
            for t in range(NT):
                src_t = xT[:, t*128:(t+1)*128].rearrange("(k p) n -> p k n",
                                                         p=128)
                if t == NT - 1:
                    # split the last tile's load so its matmuls start on the
                    # first half while the second is still in flight
                    nc.sync.dma_start(xT_sb[:, t, 0:4, :], src_t[:, 0:4, :])
                    nc.sync.dma_start(xT_sb[:, t, 4:8, :], src_t[:, 4:8, :])
                else:
                    nc.sync.dma_start(xT_sb[:, t, :, :], src_t)
                if t == 0:
                    nc.sync.dma_start(gw_sb[:], gwc[:])
            # gates (f32 bits) and idx packed in one u32 tile -> one output
            # DMA pipeline per chunk instead of two
            comb_sb = pin.tile([128, NT, 2 * K], U32)
            gout = comb_sb[:, :, 0:K].bitcast(F32)
            icoll = comb_sb[:, :, K:2*K]

            for t in range(NT):
                ps = pps.tile([128, E], F32, tag="ps")
                for k in range(8):
                    nc.tensor.matmul(ps[:], xT_sb[:, t, k, :], gw_sb[:, k, :],
                                     start=(k == 0), stop=(k == 7))
                sel = pwk.tile([128, E], F32, tag="sel")
                if bias_zero:
                    # selection key = logits (sigmoid monotone, bias 0)
                    nc.vector.tensor_copy(sel[:], ps[:])
                else:
                    # selection key = sigmoid(logits) + bias
                    nc.scalar.activation(sel[:], ps[:], AF.Sigmoid)
                    for e in range(E):
                        if float(bias_vals[e]) != 0.0:
                            nc.vector.tensor_scalar_add(
                                sel[:, e:e+1], sel[:, e:e+1], float(bias_vals[e]))
                top8 = pwk.tile([128, 8], F32, tag="top8")
                nc.vector.max(top8[:], sel[:])
                idx8 = pwk.tile([128, 8], U32, tag="idx8")
                nc.vector.max_index(idx8[:], top8[:], sel[:])
                nc.vector.tensor_copy(icoll[:, t, :], idx8[:, 0:K])
                if bias_zero:
                    nc.scalar.activation(gout[:, t, :], top8[:, 0:K], AF.Sigmoid)
                else:
                    # true score = (sigmoid+bias) - bias[selected]
                    nc.vector.tensor_copy(gout[:, t, :], top8[:, 0:K])
                    idxf = pwk.tile([128, K], F32, tag="idxf")
                    nc.vector.tensor_copy(idxf[:], idx8[:, 0:K])
                    for e in range(E):
                        if float(bias_vals[e]) == 0.0:
                            continue
                        m = pwk.tile([128, K], F32, tag="msk")
                        nc.vector.tensor_scalar(m[:], idxf[:], float(e), None,
                                                op0=ALU.is_equal)
                        nc.vector.tensor_scalar_mul(m[:], m[:],
                                                    -float(bias_vals[e]))
                        nc.vector.tensor_add(gout[:, t, :], gout[:, t, :], m[:])
                if t % 4 == 3:
                    cs = slice((t-3)*128, (t+1)*128)
                    nc.sync.dma_start(
                        comb_o[cs, :].rearrange("(t p) f -> p t f", p=128),
                        comb_sb[:, t-3:t+1, :])
    nc.compile()
    return nc


# -------------------------------------------------------------- L2: experts
def build_l2(cap):
    nc = bacc.Bacc("TRN2", target_bir_lowering=False, debug=False,
                   num_devices=NCORES)
    xgT = nc.dram_tensor("xgT", [D, cap], BF16, kind="ExternalInput").ap()
    gbr = nc.dram_tensor("gbr", [128, cap], BF16, kind="ExternalInput").ap()
    w1h = nc.dram_tensor("w1h", [8, 128, 8, 128], BF16, kind="ExternalInput").ap()
    w3h = nc.dram_tensor("w3h", [8, 128, 8, 128], BF16, kind="ExternalInput").ap()
    w2T = nc.dram_tensor("w2T", [H, D], BF16, kind="ExternalInput").ap()
    yT_o = nc.dram_tensor("yT", [D, cap], BF16, kind="ExternalOutput").ap()

    # first tile is 256 wide so the opening xgT DMA (the startup critical
    # path) is half-size; remainder lands on the last tile
    tws = [256]
    left = cap - 256
    while left > 512:
        tws.append(512)
        left -= 512
    tws.append(left)
    ntiles = len(tws)
    starts = [0]
    for w in tws[:-1]:
        starts.append(starts[-1] + w)

    with tile.TileContext(nc) as tc:
        with tc.tile_pool(name="pin", bufs=1) as pin, \
             tc.tile_pool(name="pxg", bufs=2) as pxg, \
             tc.tile_pool(name="pxs", bufs=2) as pxs, \
             tc.tile_pool(name="pgt", bufs=2) as pgt, \
             tc.tile_pool(name="pwk", bufs=2) as pwk, \
             tc.tile_pool(name="pyo", bufs=2) as pyo, \
             tc.tile_pool(name="pps", bufs=1, space="PSUM") as pps:
            gb_sb = pin.tile([128, cap], BF16)

            def load(t):
                tw = tws[t]
                cs = slice(starts[t], starts[t] + tw)
                nc.sync.dma_start(gb_sb[:, cs], gbr[:, cs])
                xg = pxg.tile([128, 8, 512], BF16, tag="xg")
                nc.sync.dma_start(
                    xg[:, :, 0:tw],
                    xgT[:, cs].rearrange("(k p) n -> p k n", p=128))
                return xg

            xg0 = load(0)
            # PE p-state warm-up: tiny matmuls on the first-arrived gb chunk
            # keep the PE busy (and the clock ramping) while weights stream
            # in; without this the first ~45 real matmuls are costed at the
            # un-ramped 1.2GHz rate.
            # warm source is an uninitialized SBUF tile: no producer, so the
            # warm-up chain starts immediately instead of waiting on a DMA
            # (values are garbage but land in an unread PSUM bank)
            wsrc = pin.tile([128, 512], BF16)
            nc.vector.memset(wsrc[:], 0.0)
            for _ in range(11):
                wp = pps.tile([128, 512], F32, tag="warm", bufs=1, name="wp")
                nc.tensor.matmul(wp[0:1, :], wsrc[:, 0:1], wsrc[:],
                                 start=True, stop=True)
            # m-major weight layout: per-m-block DMAs with 2KB descriptors so
            # the first h1 GEMM only waits on w1[m=0]; tile-1's load is
            # interleaved mid-stream so the PE (which clears the narrow
            # tile 0 quickly) never waits on it behind the weight queue
            w1r = pin.tile([128, 8, 8, 128], BF16)
            w3r = pin.tile([128, 8, 8, 128], BF16)
            for m in range(4):
                nc.sync.dma_start(w1r[:, m, :, :], w1h[m, :, :, :])
                nc.sync.dma_start(w3r[:, m, :, :], w3h[m, :, :, :])
            xg1 = load(1)
            for m in range(4, 8):
                nc.sync.dma_start(w1r[:, m, :, :], w1h[m, :, :, :])
                nc.sync.dma_start(w3r[:, m, :, :], w3h[m, :, :, :])
            w2r = pin.tile([128, 8, D], BF16)
            nc.sync.dma_start(w2r[:], w2T.rearrange("(m p) d -> p m d", p=128))

            def xscale(t, xg):
                tw = tws[t]
                cs = slice(starts[t], starts[t] + tw)
                xs = pxs.tile([128, 8, 512], BF16, tag="xs")
                for k in range(8):
                    nc.vector.tensor_mul(xs[:, k, 0:tw], xg[:, k, 0:tw],
                                         gb_sb[:, cs])
                return xs

            def hpart(t, xs, ms):
                tw = tws[t]
                gT = gts[t % 2]
                for m in ms:
                    h1 = pps.tile([128, 512], F32, tag="h1", bufs=2)
                    h3 = pps.tile([128, 512], F32, tag="h3", bufs=2)
                    for k in range(8):
                        nc.tensor.matmul(h1[:, 0:tw], w1r[:, m, k, :],
                                         xs[:, k, 0:tw],
                                         start=(k == 0), stop=(k == 7))
                    for k in range(8):
                        nc.tensor.matmul(h3[:, 0:tw], w3r[:, m, k, :],
                                         xs[:, k, 0:tw],
                                         start=(k == 0), stop=(k == 7))
                    s1 = pwk.tile([128, 512], BF16, tag="s1")
                    nc.scalar.activation(s1[:, 0:tw], h1[:, 0:tw], AF.Silu)
                    nc.vector.tensor_mul(gT[:, m, 0:tw], s1[:, 0:tw], h3[:, 0:tw])

            def ypart(t):
                tw = tws[t]
                cs = slice(starts[t], starts[t] + tw)
                last = (t == ntiles - 1)
                gT = gts[t % 2]
                yo = pyo.tile([128, 8, 512], BF16, tag="yo")
                for d in range(8):
                    yp = pps.tile([128, 512], F32, tag="yp", bufs=3)
                    for m in range(8):
                        nc.tensor.matmul(yp[:, 0:tw], w2r[:, m, d*128:(d+1)*128],
                                         gT[:, m, 0:tw],
                                         start=(m == 0), stop=(m == 7))
                    # post-scale by gate fused into the PSUM drain
                    nc.vector.tensor_mul(yo[:, d, 0:tw], yp[:, 0:tw],
                                         gb_sb[:, cs])
                    if last and d % 2 == 1:
                        nc.sync.dma_start(
                            yT_o[(d-1)*128:(d+1)*128, cs].rearrange(
                                "(d p) n -> p d n", p=128),
                            yo[:, d-1:d+1, 0:tw])
                    elif not last and d == 3:
                        nc.sync.dma_start(
                            yT_o[0:512, cs].rearrange(
                                "(d p) n -> p d n", p=128),
                            yo[:, 0:4, 0:tw])
                if not last:
                    nc.sync.dma_start(
                        yT_o[512:1024, cs].rearrange(
                            "(d p) n -> p d n", p=128),
                        yo[:, 4:8, 0:tw])

            gts = [pgt.tile([128, 8, 512], BF16, tag="gTa", name="gTa"),
                   pgt.tile([128, 8, 512], BF16, tag="gTb", name="gTb")]

            xs = xscale(0, xg0)
            hpart(0, xs, range(8))
            for t in range(1, ntiles):
                xg = xg1 if t == 1 else load(t)
                xs = xscale(t, xg)
                # interleave w2 GEMM of tile t-1 inside w1/w3 GEMMs of tile t
                hpart(t, xs, range(4))
                ypart(t - 1)
                hpart(t, xs, range(4, 8))
            ypart(ntiles - 1)
    nc.compile()
    return nc


# ------------------------------------------------------ L3: shared + combine
def build_l3():
    nc = bacc.Bacc("TRN2", target_bir_lowering=False, debug=False,
                   num_devices=NCORES)
    xTr = nc.dram_tensor("xTr", [D, TPC], BF16, kind="ExternalInput").ap()
    sw1h = nc.dram_tensor("sw1h", [8, 128, 8, 128], BF16, kind="ExternalInput").ap()
    sw3h = nc.dram_tensor("sw3h", [8, 128, 8, 128], BF16, kind="ExternalInput").ap()
    sw2T = nc.dram_tensor("sw2T", [H, D], BF16, kind="ExternalInput").ap()
    AT = nc.dram_tensor("AT", [D, TPC], BF16, kind="ExternalInput").ap()
    BT = nc.dram_tensor("BT", [D, TPC], BF16, kind="ExternalInput").ap()
    warm = nc.dram_tensor("warm", [128, 512], BF16, kind="ExternalInput").ap()
    outT_o = nc.dram_tensor("outT", [D, TPC], F32, kind="ExternalOutput").ap()

    nh = TPC // 512

    with tile.TileContext(nc) as tc:
        with tc.tile_pool(name="pin", bufs=1) as pin, \
             tc.tile_pool(name="pgt", bufs=2) as pgt, \
             tc.tile_pool(name="pwk", bufs=2) as pwk, \
             tc.tile_pool(name="pos", bufs=2) as pos, \
             tc.tile_pool(name="pab", bufs=2) as pab, \
             tc.tile_pool(name="pps", bufs=1, space="PSUM") as pps:
            wm_sb = pin.tile([128, 512], BF16)
            nc.sync.dma_start(wm_sb[:], warm[:])
            xT_sb = pin.tile([128, 8, TPC], BF16)
            nc.sync.dma_start(
                xT_sb[:, :, 0:512],
                xTr[:, 0:512].rearrange("(k p) n -> p k n", p=128))
            # PE p-state warm-up (see L2)
            for _ in range(13):
                wp = pps.tile([128, 512], F32, tag="warm", bufs=1, name="wp")
                nc.tensor.matmul(wp[0:1, :], wm_sb[:, 0:1], wm_sb[:],
                                 start=True, stop=True)
            w1r = pin.tile([128, 8, 8, 128], BF16)
            w3r = pin.tile([128, 8, 8, 128], BF16)
            for m in range(8):
                nc.sync.dma_start(w1r[:, m, :, :], sw1h[m, :, :, :])
                nc.sync.dma_start(w3r[:, m, :, :], sw3h[m, :, :, :])
            for hh in range(1, nh):
                nc.sync.dma_start(
                    xT_sb[:, :, hh*512:(hh+1)*512],
                    xTr[:, hh*512:(hh+1)*512].rearrange("(k p) n -> p k n", p=128))
            w2r = pin.tile([128, 8, D], BF16)
            nc.sync.dma_start(w2r[:], sw2T.rearrange("(m p) d -> p m d", p=128))
            at_sb = pin.tile([128, 8, TPC], BF16)
            bt_sb = pin.tile([128, 8, TPC], BF16)
            for hh in range(nh):
                cs = slice(hh*512, (hh+1)*512)
                nc.sync.dma_start(at_sb[:, :, cs],
                                  AT[:, cs].rearrange("(d p) n -> p d n", p=128))
                nc.sync.dma_start(bt_sb[:, :, cs],
                                  BT[:, cs].rearrange("(d p) n -> p d n", p=128))

            gts = [pgt.tile([128, 8, 512], BF16, tag="gTa", name="gTa"),
                   pgt.tile([128, 8, 512], BF16, tag="gTb", name="gTb")]

            def hpart(hh, ms, split_first=False):
                toks = slice(hh*512, (hh+1)*512)
                gT = gts[hh % 2]
                for m in ms:
                    h1 = pps.tile([128, 512], F32, tag="h1", bufs=2)
                    h3 = pps.tile([128, 512], F32, tag="h3", bufs=2)
                    if split_first and m == ms[0]:
                        for half in range(2):
                            cls = slice(half*256, (half+1)*256)
                            for k in range(8):
                                nc.tensor.matmul(
                                    h1[:, cls], w1r[:, m, k, :],
                                    xT_sb[:, k, hh*512+half*256:
                                          hh*512+(half+1)*256],
                                    start=(k == 0), stop=(k == 7))
                    else:
                        for k in range(8):
                            nc.tensor.matmul(h1[:], w1r[:, m, k, :],
                                             xT_sb[:, k, toks],
                                             start=(k == 0), stop=(k == 7))
                    for k in range(8):
                        nc.tensor.matmul(h3[:], w3r[:, m, k, :],
                                         xT_sb[:, k, toks],
                                         start=(k == 0), stop=(k == 7))
                    s1 = pwk.tile([128, 512], BF16, tag="s1")
                    nc.scalar.activation(s1[:], h1[:], AF.Silu)
                    nc.vector.tensor_mul(gT[:, m, :], s1[:], h3[:])

            def absum(hh):
                # A+B pre-sum on DVE slack so the combine is one op per block
                toks = slice(hh*512, (hh+1)*512)
                ab = pab.tile([128, 8, 512], BF16, tag="ab")
                for d in range(8):
                    nc.vector.tensor_add(ab[:, d, :], at_sb[:, d, toks],
                                         bt_sb[:, d, toks])
                return ab

            def ypart(hh, ab):
                toks = slice(hh*512, (hh+1)*512)
                last = (hh == nh - 1)
                gT = gts[hh % 2]
                out_sb = pos.tile([128, 8, 512], F32, tag="os")
                for d in range(8):
                    yp = pps.tile([128, 512], F32, tag="yp", bufs=3)
                    for m in range(8):
                        nc.tensor.matmul(yp[:], w2r[:, m, d*128:(d+1)*128],
                                         gT[:, m, :],
                                         start=(m == 0), stop=(m == 7))
                    # combine: out = shared + (A + B), straight off PSUM
                    nc.vector.scalar_tensor_tensor(
                        out_sb[:, d, :], yp[:], 1.0, ab[:, d, :],
                        op0=ALU.mult, op1=ALU.add)
                    # drain: on the last half, shrink the final chunks so the
                    # tail after the last matmul is just d7's DMA
                    if last:
                        chunk = {1: 0, 3: 2, 5: 4, 6: 6, 7: 7}.get(d)
                    else:
                        chunk = d - 1 if d % 2 == 1 else None
                    if chunk is not None:
                        nc.sync.dma_start(
                            outT_o[chunk*128:(d+1)*128, toks].rearrange(
                                "(d p) n -> p d n", p=128),
                            out_sb[:, chunk:d+1, :])

            hpart(0, range(8), split_first=True)
            ab = absum(0)
            for hh in range(1, nh):
                hpart(hh, range(4))
                ypart(hh - 1, ab)
                hpart(hh, range(4, 8))
                ab = absum(hh)
            ypart(nh - 1, ab)
    nc.compile()
    return nc


_BUILT = {}


def _get(name, builder, *args):
    key = (name,) + tuple(args)
    if key not in _BUILT:
        _BUILT[key] = builder(*args)
    return _BUILT[key], key


def kernel(**inputs):
    x = np.ascontiguousarray(np.asarray(inputs["x"], dtype=np.float32))
    xf = x.reshape(T, D)
    gw = np.asarray(inputs["gate_w"], dtype=np.float32)
    bias = np.asarray(inputs["expert_bias"], dtype=np.float32)
    w1 = np.asarray(inputs["w1"], dtype=np.float32)
    w2 = np.asarray(inputs["w2"], dtype=np.float32)
    w3 = np.asarray(inputs["w3"], dtype=np.float32)
    sw1 = np.asarray(inputs["sw1"], dtype=np.float32)
    sw2 = np.asarray(inputs["sw2"], dtype=np.float32)
    sw3 = np.asarray(inputs["sw3"], dtype=np.float32)

    cores = list(range(NCORES))

    # ---- L1 router ----
    nc1, _ = _get("l1", build_l1, tuple(float(b) for b in bias))
    gwc = np.ascontiguousarray(gw.T.reshape(8, 128, 8).transpose(1, 0, 2))
    in1 = [{"xT": np.ascontiguousarray(xf[c*TPC:(c+1)*TPC].T), "gwc": gwc}
           for c in cores]
    r1 = run_bass_kernel_spmd(nc1, in1, cores).results
    comb = np.concatenate([r["comb"] for r in r1])        # [T, 4] uint32
    gates = np.ascontiguousarray(comb[:, 0:K]).view(np.float32)  # [T, 2]
    sel = comb[:, K:2*K]                                  # [T, 2] uint32

    # ---- host dispatch (pure permutation / layout) ----
    flat_sel = sel.reshape(-1).astype(np.int64)
    order = np.argsort(flat_sel, kind="stable")
    counts = np.bincount(flat_sel, minlength=E)
    offs = np.zeros(E + 1, np.int64)
    np.cumsum(counts, out=offs[1:])
    cap = max(512, int(counts.max()))
    gflat = gates.reshape(-1)
    xf_bf = xf.astype(NPBF16)

    slots_e = [order[offs[e]:offs[e+1]] for e in range(E)]
    in2 = []
    for e in cores:
        n = counts[e]
        slots = slots_e[e]
        toks = slots >> 1
        xg = np.zeros((cap, D), NPBF16)
        xg[:n] = xf_bf[toks]
        gbrow = np.zeros((cap,), NPBF16)
        gbrow[:n] = gflat[slots].astype(NPBF16)
        in2.append({
            "xgT": np.ascontiguousarray(xg.T),
            "gbr": np.ascontiguousarray(np.broadcast_to(gbrow, (128, cap))),
            "w1h": _mmajor(w1[e].T),
            "w3h": _mmajor(w3[e].T),
            "w2T": np.ascontiguousarray(w2[e].T.astype(NPBF16)),
        })

    nc2, _ = _get("l2", build_l2, cap)
    r2 = run_bass_kernel_spmd(nc2, in2, cores).results

    # ---- host re-layout of routed contributions (pure permutation) ----
    ATfull = np.zeros((D, T), NPBF16)
    BTfull = np.zeros((D, T), NPBF16)
    total_valid = 0
    for e in cores:
        n = counts[e]
        slots = slots_e[e]
        toks = slots >> 1
        kk = (slots & 1).astype(bool)
        yT = r2[e]["yT"]                         # [D, cap] bf16
        ATfull[:, toks[~kk]] = yT[:, :n][:, ~kk]
        BTfull[:, toks[kk]] = yT[:, :n][:, kk]
        total_valid += n
    assert total_valid == T * K, f"dropped slots: {total_valid} != {T*K}"

    # ---- L3 shared + combine ----
    nc3, _ = _get("l3", build_l3)
    sw1h = _mmajor(sw1.T)
    sw3h = _mmajor(sw3.T)
    sw2T = np.ascontiguousarray(sw2.T.astype(NPBF16))
    in3 = []
    for c in cores:
        sl = slice(c*TPC, (c+1)*TPC)
        in3.append({
            "xTr": np.ascontiguousarray(xf_bf[sl].T),
            "sw1h": sw1h, "sw3h": sw3h, "sw2T": sw2T,
            "AT": np.ascontiguousarray(ATfull[:, sl]),
            "BT": np.ascontiguousarray(BTfull[:, sl]),
            "warm": np.zeros((128, 512), NPBF16),
        })
    r3 = run_bass_kernel_spmd(nc3, in3, cores).results
    out = np.concatenate([r["outT"].T for r in r3])
    return np.ascontiguousarray(out).reshape(x.shape).astype(
        inputs["x"].dtype, copy=False)


# revision 26
# speedup vs baseline: 1.0057x; 1.0033x over previous
"""MoE routing kernel for 8 Trainium2 NeuronCores.

Strategy (expert-parallel, 3 launches; host does only data movement):
  L1  router   : data-parallel over tokens. Exact-fp32 gate matmul in
                 token-partition orientation (out free dim = 8 experts, so
                 the fp32 4x penalty is negligible), top-2 via DVE
                 max/max_index on logits (sigmoid monotone; bias path when
                 expert_bias != 0), per-tile sigmoid, batched output DMAs.
  L2  experts  : one expert per core. Host gathers + transposes that
                 expert's token rows to [D, CAP] bf16 and replicates the
                 gate row to [128, CAP]; device pre-scales by gate on DVE,
                 runs the GLU MLP as pure bf16 GEMMs (no on-device
                 transposes or gathers), and fuses the post-scale into the
                 PSUM->bf16 drain. Weights arrive as per-m-block DMAs in
                 m-major host layout so the first GEMM starts ~6us in; the
                 w2 GEMM of tile t-1 is interleaved inside the w1/w3 GEMMs
                 of tile t so the PE never stalls on the gT latency.
  L3  combine  : data-parallel over token slices. Shared-expert GLU MLP in
                 bf16, combine = two DVE adds of host-retransposed routed
                 contributions (AT/BT, [D, TPC] bf16) directly on the w2
                 PSUM output; result stays [D, TPC] f32 (host transposes
                 back), output drained in half-chunks to overlap the final
                 DMA with compute.
"""
import sys
sys.path.insert(0, '/opt/trn_rl_repo')

import numpy as np
import ml_dtypes

import concourse.bacc as bacc
import concourse.mybir as mybir
import concourse.tile as tile
from concourse.bass_utils import run_bass_kernel_spmd

F32 = mybir.dt.float32
BF16 = mybir.dt.bfloat16
U32 = mybir.dt.uint32
AF = mybir.ActivationFunctionType
ALU = mybir.AluOpType
NPBF16 = ml_dtypes.bfloat16

NCORES = 8
E = 8           # experts
K = 2           # top-k
D = 1024
H = 1024
T = 8192        # total tokens (B*S)
TPC = T // NCORES   # tokens per core (router / combine slices)


def _mmajor(wT):
    """[D, H] f32 -> [8(m), 128(p), 8(k), 128(j)] bf16 contiguous, so a
    per-m-block DMA moves 2KB-contiguous rows: w[m, p, k, j] = wT[k*128+p,
    m*128+j]."""
    return np.ascontiguousarray(
        wT.reshape(8, 128, 8, 128).transpose(2, 1, 0, 3).astype(NPBF16))


# --------------------------------------------------------------- L1: router
def build_l1(bias_vals):
    nc = bacc.Bacc("TRN2", target_bir_lowering=False, debug=False,
                   num_devices=NCORES)
    xT = nc.dram_tensor("xT", [D, TPC], F32, kind="ExternalInput").ap()
    gwc = nc.dram_tensor("gwc", [128, 8, E], F32, kind="ExternalInput").ap()
    comb_o = nc.dram_tensor("comb", [TPC, 2 * K], U32, kind="ExternalOutput").ap()
    bias_zero = all(float(b) == 0.0 for b in bias_vals)
    NT = TPC // 128

    with tile.TileContext(nc) as tc:
        with tc.tile_pool(name="pin", bufs=1) as pin, \
             tc.tile_pool(name="pps", bufs=4, space="PSUM") as pps, \
             tc.tile_pool(name="pwk", bufs=4) as pwk:
            xT_sb = pin.tile([128, NT, 8, 128], F32)
            gw_sb = pin.tile([128, 8, E], F32)
            for t in range(NT):
                src_t = xT[:, t*128:(t+1)*128].rearrange("(k p) n -> p k n",
                                                         p=128)
                if t == NT - 1:
                    # split the last tile's load so its matmuls start on the
                    # first half while the second is still in flight
                    nc.sync.dma_start(xT_sb[:, t, 0:4, :], src_t[:, 0:4, :])
                    nc.sync.dma_start(xT_sb[:, t, 4:8, :], src_t[:, 4:8, :])
                else:
                    nc.sync.dma_start(xT_sb[:, t, :, :], src_t)
                if t == 0:
                    nc.sync.dma_start(gw_sb[:], gwc[:])
            # gates (f32 bits) and idx packed in one u32 tile -> one output
            # DMA pipeline per chunk instead of two
            comb_sb = pin.tile([128, NT, 2 * K], U32)
            gout = comb_sb[:, :, 0:K].bitcast(F32)
            icoll = comb_sb[:, :, K:2*K]

            for t in range(NT):
                ps = pps.tile([128, E], F32, tag="ps")
                for k in range(8):
                    nc.tensor.matmul(ps[:], xT_sb[:, t, k, :], gw_sb[:, k, :],
                                     start=(k == 0), stop=(k == 7))
                sel = pwk.tile([128, E], F32, tag="sel")
                if bias_zero:
                    # selection key = logits (sigmoid monotone, bias 0)
                    nc.vector.tensor_copy(sel[:], ps[:])
                else:
                    # selection key = sigmoid(logits) + bias
                    nc.scalar.activation(sel[:], ps[:], AF.Sigmoid)
                    for e in range(E):
                        if float(bias_vals[e]) != 0.0:
                            nc.vector.tensor_scalar_add(
                                sel[:, e:e+1], sel[:, e:e+1], float(bias_vals[e]))
                top8 = pwk.tile([128, 8], F32, tag="top8")
                nc.vector.max(top8[:], sel[:])
                idx8 = pwk.tile([128, 8], U32, tag="idx8")
                nc.vector.max_index(idx8[:], top8[:], sel[:])
                nc.vector.tensor_copy(icoll[:, t, :], idx8[:, 0:K])
                if bias_zero:
                    nc.scalar.activation(gout[:, t, :], top8[:, 0:K], AF.Sigmoid)
                else:
                    # true score = (sigmoid+bias) - bias[selected]
                    nc.vector.tensor_copy(gout[:, t, :], top8[:, 0:K])
                    idxf = pwk.tile([128, K], F32, tag="idxf")
                    nc.vector.tensor_copy(idxf[:], idx8[:, 0:K])
                    for e in range(E):
                        if float(bias_vals[e]) == 0.0:
                            continue
                        m = pwk.tile([128, K], F32, tag="msk")
                        nc.vector.tensor_scalar(m[:], idxf[:], float(e), None,
                                                op0=ALU.is_equal)
                        nc.vector.tensor_scalar_mul(m[:], m[:],
                                                    -float(bias_vals[e]))
                        nc.vector.tensor_add(gout[:, t, :], gout[:, t, :], m[:])
                if t % 4 == 3:
                    cs = slice((t-3)*128, (t+1)*128)
                    nc.sync.dma_start(
                        comb_o[cs, :].rearrange("(t p) f -> p t f", p=128),
                        comb_sb[:, t-3:t+1, :])
    nc.compile()
    return nc


# -------------------------------------------------------------- L2: experts
def build_l2(cap):
    nc = bacc.Bacc("TRN2", target_bir_lowering=False, debug=False,
                   num_devices=NCORES)
    xgT = nc.dram_tensor("xgT", [D, cap], BF16, kind="ExternalInput").ap()
    gbr = nc.dram_tensor("gbr", [128, cap], BF16, kind="ExternalInput").ap()
    w1h = nc.dram_tensor("w1h", [8, 128, 8, 128], BF16, kind="ExternalInput").ap()
    w3h = nc.dram_tensor("w3h", [8, 128, 8, 128], BF16, kind="ExternalInput").ap()
    w2T = nc.dram_tensor("w2T", [H, D], BF16, kind="ExternalInput").ap()
    yT_o = nc.dram_tensor("yT", [D, cap], BF16, kind="ExternalOutput").ap()

    # first tile is 256 wide so the opening xgT DMA (the startup critical
    # path) is half-size; remainder lands on the last tile
    tws = [256]
    left = cap - 256
    while left > 512:
        tws.append(512)
        left -= 512
    tws.append(left)
    ntiles = len(tws)
    starts = [0]
    for w in tws[:-1]:
        starts.append(starts[-1] + w)

    with tile.TileContext(nc) as tc:
        with tc.tile_pool(name="pin", bufs=1) as pin, \
             tc.tile_pool(name="pxg", bufs=2) as pxg, \
             tc.tile_pool(name="pxs", bufs=2) as pxs, \
             tc.tile_pool(name="pgt", bufs=2) as pgt, \
             tc.tile_pool(name="pwk", bufs=2) as pwk, \
             tc.tile_pool(name="pyo", bufs=2) as pyo, \
             tc.tile_pool(name="pps", bufs=1, space="PSUM") as pps:
            gb_sb = pin.tile([128, cap], BF16)

            def load(t):
                tw = tws[t]
                cs = slice(starts[t], starts[t] + tw)
                nc.sync.dma_start(gb_sb[:, cs], gbr[:, cs])
                xg = pxg.tile([128, 8, 512], BF16, tag="xg")
                nc.sync.dma_start(
                    xg[:, :, 0:tw],
                    xgT[:, cs].rearrange("(k p) n -> p k n", p=128))
                return xg

            xg0 = load(0)
            # PE p-state warm-up: tiny matmuls on the first-arrived gb chunk
            # keep the PE busy (and the clock ramping) while weights stream
            # in; without this the first ~45 real matmuls are costed at the
            # un-ramped 1.2GHz rate.
            # warm source is an uninitialized SBUF tile: no producer, so the
            # warm-up chain starts immediately instead of waiting on a DMA
            # (values are garbage but land in an unread PSUM bank)
            wsrc = pin.tile([128, 512], BF16)
            nc.vector.memset(wsrc[:], 0.0)
            for _ in range(11):
                wp = pps.tile([128, 512], F32, tag="warm", bufs=1, name="wp")
                nc.tensor.matmul(wp[0:1, :], wsrc[:, 0:1], wsrc[:],
                                 start=True, stop=True)
            # m-major weight layout: per-m-block DMAs with 2KB descriptors so
            # the first h1 GEMM only waits on w1[m=0]; tile-1's load is
            # interleaved mid-stream so the PE (which clears the narrow
            # tile 0 quickly) never waits on it behind the weight queue
            w1r = pin.tile([128, 8, 8, 128], BF16)
            w3r = pin.tile([128, 8, 8, 128], BF16)
            for m in range(8):
                nc.sync.dma_start(w1r[:, m, :, :], w1h[m, :, :, :])
                nc.sync.dma_start(w3r[:, m, :, :], w3h[m, :, :, :])
            xg1 = load(1)
            w2r = pin.tile([128, 8, D], BF16)
            nc.sync.dma_start(w2r[:], w2T.rearrange("(m p) d -> p m d", p=128))

            def xscale(t, xg):
                tw = tws[t]
                cs = slice(starts[t], starts[t] + tw)
                xs = pxs.tile([128, 8, 512], BF16, tag="xs")
                for k in range(8):
                    nc.vector.tensor_mul(xs[:, k, 0:tw], xg[:, k, 0:tw],
                                         gb_sb[:, cs])
                return xs

            def hpart(t, xs, ms):
                tw = tws[t]
                gT = gts[t % 2]
                for m in ms:
                    h1 = pps.tile([128, 512], F32, tag="h1", bufs=2)
                    h3 = pps.tile([128, 512], F32, tag="h3", bufs=2)
                    for k in range(8):
                        nc.tensor.matmul(h1[:, 0:tw], w1r[:, m, k, :],
                                         xs[:, k, 0:tw],
                                         start=(k == 0), stop=(k == 7))
                    for k in range(8):
                        nc.tensor.matmul(h3[:, 0:tw], w3r[:, m, k, :],
                                         xs[:, k, 0:tw],
                                         start=(k == 0), stop=(k == 7))
                    s1 = pwk.tile([128, 512], BF16, tag="s1")
                    nc.scalar.activation(s1[:, 0:tw], h1[:, 0:tw], AF.Silu)
                    nc.vector.tensor_mul(gT[:, m, 0:tw], s1[:, 0:tw], h3[:, 0:tw])

            def ypart(t):
                tw = tws[t]
                cs = slice(starts[t], starts[t] + tw)
                last = (t == ntiles - 1)
                gT = gts[t % 2]
                yo = pyo.tile([128, 8, 512], BF16, tag="yo")
                for d in range(8):
                    yp = pps.tile([128, 512], F32, tag="yp", bufs=3)
                    for m in range(8):
                        nc.tensor.matmul(yp[:, 0:tw], w2r[:, m, d*128:(d+1)*128],
                                         gT[:, m, 0:tw],
                                         start=(m == 0), stop=(m == 7))
                    # post-scale by gate fused into the PSUM drain
                    nc.vector.tensor_mul(yo[:, d, 0:tw], yp[:, 0:tw],
                                         gb_sb[:, cs])
                    if last and d % 2 == 1:
                        nc.sync.dma_start(
                            yT_o[(d-1)*128:(d+1)*128, cs].rearrange(
                                "(d p) n -> p d n", p=128),
                            yo[:, d-1:d+1, 0:tw])
                    elif not last and d == 3:
                        nc.sync.dma_start(
                            yT_o[0:512, cs].rearrange(
                                "(d p) n -> p d n", p=128),
                            yo[:, 0:4, 0:tw])
                if not last:
                    nc.sync.dma_start(
                        yT_o[512:1024, cs].rearrange(
                            "(d p) n -> p d n", p=128),
                        yo[:, 4:8, 0:tw])

            gts = [pgt.tile([128, 8, 512], BF16, tag="gTa", name="gTa"),
                   pgt.tile([128, 8, 512], BF16, tag="gTb", name="gTb")]

            xs = xscale(0, xg0)
            hpart(0, xs, range(8))
            for t in range(1, ntiles):
                xg = xg1 if t == 1 else load(t)
                xs = xscale(t, xg)
                # interleave w2 GEMM of tile t-1 inside w1/w3 GEMMs of tile t
                hpart(t, xs, range(4))
                ypart(t - 1)
                hpart(t, xs, range(4, 8))
            ypart(ntiles - 1)
    nc.compile()
    return nc


# ------------------------------------------------------ L3: shared + combine
def build_l3():
    nc = bacc.Bacc("TRN2", target_bir_lowering=False, debug=False,
                   num_devices=NCORES)
    xTr = nc.dram_tensor("xTr", [D, TPC], BF16, kind="ExternalInput").ap()
    sw1h = nc.dram_tensor("sw1h", [8, 128, 8, 128], BF16, kind="ExternalInput").ap()
    sw3h = nc.dram_tensor("sw3h", [8, 128, 8, 128], BF16, kind="ExternalInput").ap()
    sw2T = nc.dram_tensor("sw2T", [H, D], BF16, kind="ExternalInput").ap()
    AT = nc.dram_tensor("AT", [D, TPC], BF16, kind="ExternalInput").ap()
    BT = nc.dram_tensor("BT", [D, TPC], BF16, kind="ExternalInput").ap()
    warm = nc.dram_tensor("warm", [128, 512], BF16, kind="ExternalInput").ap()
    outT_o = nc.dram_tensor("outT", [D, TPC], F32, kind="ExternalOutput").ap()

    nh = TPC // 512

    with tile.TileContext(nc) as tc:
        with tc.tile_pool(name="pin", bufs=1) as pin, \
             tc.tile_pool(name="pgt", bufs=2) as pgt, \
             tc.tile_pool(name="pwk", bufs=2) as pwk, \
             tc.tile_pool(name="pos", bufs=2) as pos, \
             tc.tile_pool(name="pab", bufs=2) as pab, \
             tc.tile_pool(name="pps", bufs=1, space="PSUM") as pps:
            wm_sb = pin.tile([128, 512], BF16)
            nc.sync.dma_start(wm_sb[:], warm[:])
            xT_sb = pin.tile([128, 8, TPC], BF16)
            nc.sync.dma_start(
                xT_sb[:, :, 0:512],
                xTr[:, 0:512].rearrange("(k p) n -> p k n", p=128))
            # PE p-state warm-up (see L2)
            for _ in range(13):
                wp = pps.tile([128, 512], F32, tag="warm", bufs=1, name="wp")
                nc.tensor.matmul(wp[0:1, :], wm_sb[:, 0:1], wm_sb[:],
                                 start=True, stop=True)
            w1r = pin.tile([128, 8, 8, 128], BF16)
            w3r = pin.tile([128, 8, 8, 128], BF16)
            for m in range(8):
                nc.sync.dma_start(w1r[:, m, :, :], sw1h[m, :, :, :])
                nc.sync.dma_start(w3r[:, m, :, :], sw3h[m, :, :, :])
            for hh in range(1, nh):
                nc.sync.dma_start(
                    xT_sb[:, :, hh*512:(hh+1)*512],
                    xTr[:, hh*512:(hh+1)*512].rearrange("(k p) n -> p k n", p=128))
            w2r = pin.tile([128, 8, D], BF16)
            nc.sync.dma_start(w2r[:], sw2T.rearrange("(m p) d -> p m d", p=128))
            at_sb = pin.tile([128, 8, TPC], BF16)
            bt_sb = pin.tile([128, 8, TPC], BF16)
            for hh in range(nh):
                cs = slice(hh*512, (hh+1)*512)
                nc.sync.dma_start(at_sb[:, :, cs],
                                  AT[:, cs].rearrange("(d p) n -> p d n", p=128))
                nc.sync.dma_start(bt_sb[:, :, cs],
                                  BT[:, cs].rearrange("(d p) n -> p d n", p=128))

            gts = [pgt.tile([128, 8, 512], BF16, tag="gTa", name="gTa"),
                   pgt.tile([128, 8, 512], BF16, tag="gTb", name="gTb")]

            def hpart(hh, ms, split_first=False):
                toks = slice(hh*512, (hh+1)*512)
                gT = gts[hh % 2]
                for m in ms:
                    h1 = pps.tile([128, 512], F32, tag="h1", bufs=2)
                    h3 = pps.tile([128, 512], F32, tag="h3", bufs=2)
                    if split_first and m == ms[0]:
                        for half in range(2):
                            cls = slice(half*256, (half+1)*256)
                            for k in range(8):
                                nc.tensor.matmul(
                                    h1[:, cls], w1r[:, m, k, :],
                                    xT_sb[:, k, hh*512+half*256:
                                          hh*512+(half+1)*256],
                                    start=(k == 0), stop=(k == 7))
                    else:
                        for k in range(8):
                            nc.tensor.matmul(h1[:], w1r[:, m, k, :],
                                             xT_sb[:, k, toks],
                                             start=(k == 0), stop=(k == 7))
                    for k in range(8):
                        nc.tensor.matmul(h3[:], w3r[:, m, k, :],
                                         xT_sb[:, k, toks],
                                         start=(k == 0), stop=(k == 7))
                    s1 = pwk.tile([128, 512], BF16, tag="s1")
                    nc.scalar.activation(s1[:], h1[:], AF.Silu)
                    nc.vector.tensor_mul(gT[:, m, :], s1[:], h3[:])

            def absum(hh):
                # A+B pre-sum on DVE slack so the combine is one op per block
                toks = slice(hh*512, (hh+1)*512)
                ab = pab.tile([128, 8, 512], BF16, tag="ab")
                for d in range(8):
                    nc.vector.tensor_add(ab[:, d, :], at_sb[:, d, toks],
                                         bt_sb[:, d, toks])
                return ab

            def ypart(hh, ab):
                toks = slice(hh*512, (hh+1)*512)
                last = (hh == nh - 1)
                gT = gts[hh % 2]
                out_sb = pos.tile([128, 8, 512], F32, tag="os")
                for d in range(8):
                    yp = pps.tile([128, 512], F32, tag="yp", bufs=3)
                    for m in range(8):
                        nc.tensor.matmul(yp[:], w2r[:, m, d*128:(d+1)*128],
                                         gT[:, m, :],
                                         start=(m == 0), stop=(m == 7))
                    # combine: out = shared + (A + B), straight off PSUM
                    nc.vector.scalar_tensor_tensor(
                        out_sb[:, d, :], yp[:], 1.0, ab[:, d, :],
                        op0=ALU.mult, op1=ALU.add)
                    # drain: on the last half, shrink the final chunks so the
                    # tail after the last matmul is just d7's DMA
                    if last:
                        chunk = {1: 0, 3: 2, 5: 4, 6: 6, 7: 7}.get(d)
                    else:
                        chunk = d - 1 if d % 2 == 1 else None
                    if chunk is not None:
                        nc.sync.dma_start(
                            outT_o[chunk*128:(d+1)*128, toks].rearrange(
                                "(d p) n -> p d n", p=128),
                            out_sb[:, chunk:d+1, :])

            hpart(0, range(8), split_first=True)
            ab = absum(0)
            for hh in range(1, nh):
                hpart(hh, range(4))
                ypart(hh - 1, ab)
                hpart(hh, range(4, 8))
                ab = absum(hh)
            ypart(nh - 1, ab)
    nc.compile()
    return nc


_BUILT = {}


def _get(name, builder, *args):
    key = (name,) + tuple(args)
    if key not in _BUILT:
        _BUILT[key] = builder(*args)
    return _BUILT[key], key


def kernel(**inputs):
    x = np.ascontiguousarray(np.asarray(inputs["x"], dtype=np.float32))
    xf = x.reshape(T, D)
    gw = np.asarray(inputs["gate_w"], dtype=np.float32)
    bias = np.asarray(inputs["expert_bias"], dtype=np.float32)
    w1 = np.asarray(inputs["w1"], dtype=np.float32)
    w2 = np.asarray(inputs["w2"], dtype=np.float32)
    w3 = np.asarray(inputs["w3"], dtype=np.float32)
    sw1 = np.asarray(inputs["sw1"], dtype=np.float32)
    sw2 = np.asarray(inputs["sw2"], dtype=np.float32)
    sw3 = np.asarray(inputs["sw3"], dtype=np.float32)

    cores = list(range(NCORES))

    # ---- L1 router ----
    nc1, _ = _get("l1", build_l1, tuple(float(b) for b in bias))
    gwc = np.ascontiguousarray(gw.T.reshape(8, 128, 8).transpose(1, 0, 2))
    in1 = [{"xT": np.ascontiguousarray(xf[c*TPC:(c+1)*TPC].T), "gwc": gwc}
           for c in cores]
    r1 = run_bass_kernel_spmd(nc1, in1, cores).results
    comb = np.concatenate([r["comb"] for r in r1])        # [T, 4] uint32
    gates = np.ascontiguousarray(comb[:, 0:K]).view(np.float32)  # [T, 2]
    sel = comb[:, K:2*K]                                  # [T, 2] uint32

    # ---- host dispatch (pure permutation / layout) ----
    flat_sel = sel.reshape(-1).astype(np.int64)
    order = np.argsort(flat_sel, kind="stable")
    counts = np.bincount(flat_sel, minlength=E)
    offs = np.zeros(E + 1, np.int64)
    np.cumsum(counts, out=offs[1:])
    cap = max(512, int(counts.max()))
    gflat = gates.reshape(-1)
    xf_bf = xf.astype(NPBF16)

    slots_e = [order[offs[e]:offs[e+1]] for e in range(E)]
    in2 = []
    for e in cores:
        n = counts[e]
        slots = slots_e[e]
        toks = slots >> 1
        xg = np.zeros((cap, D), NPBF16)
        xg[:n] = xf_bf[toks]
        gbrow = np.zeros((cap,), NPBF16)
        gbrow[:n] = gflat[slots].astype(NPBF16)
        in2.append({
            "xgT": np.ascontiguousarray(xg.T),
            "gbr": np.ascontiguousarray(np.broadcast_to(gbrow, (128, cap))),
            "w1h": _mmajor(w1[e].T),
            "w3h": _mmajor(w3[e].T),
            "w2T": np.ascontiguousarray(w2[e].T.astype(NPBF16)),
        })

    nc2, _ = _get("l2", build_l2, cap)
    r2 = run_bass_kernel_spmd(nc2, in2, cores).results

    # ---- host re-layout of routed contributions (pure permutation) ----
    ATfull = np.zeros((D, T), NPBF16)
    BTfull = np.zeros((D, T), NPBF16)
    total_valid = 0
    for e in cores:
        n = counts[e]
        slots = slots_e[e]
        toks = slots >> 1
        kk = (slots & 1).astype(bool)
        yT = r2[e]["yT"]                         # [D, cap] bf16
        ATfull[:, toks[~kk]] = yT[:, :n][:, ~kk]
        BTfull[:, toks[kk]] = yT[:, :n][:, kk]
        total_valid += n
    assert total_valid == T * K, f"dropped slots: {total_valid} != {T*K}"

    # ---- L3 shared + combine ----
    nc3, _ = _get("l3", build_l3)
    sw1h = _mmajor(sw1.T)
    sw3h = _mmajor(sw3.T)
    sw2T = np.ascontiguousarray(sw2.T.astype(NPBF16))
    in3 = []
    for c in cores:
        sl = slice(c*TPC, (c+1)*TPC)
        in3.append({
            "xTr": np.ascontiguousarray(xf_bf[sl].T),
            "sw1h": sw1h, "sw3h": sw3h, "sw2T": sw2T,
            "AT": np.ascontiguousarray(ATfull[:, sl]),
            "BT": np.ascontiguousarray(BTfull[:, sl]),
            "warm": np.zeros((128, 512), NPBF16),
        })
    r3 = run_bass_kernel_spmd(nc3, in3, cores).results
    out = np.concatenate([r["outT"].T for r in r3])
    return np.ascontiguousarray(out).reshape(x.shape).astype(
        inputs["x"].dtype, copy=False)
